# revision 4
# baseline (speedup 1.0000x reference)
"""TRN2 Bass kernel for nn_HCSMoEQwen3MoeSparseMoeBlock (8-core expert-parallel).

Reference semantics: router softmax over 32 experts -> top-8 -> normalized
per-(token,group) weights via merge_groups; every token is processed by the 8
groups' dominant experts (SwiGLU MLPs); outputs combined with the weights.

Sharding: core g owns group g's dominant expert (gate_up/down weights for
that expert only) and processes ALL tokens; the router is replicated (each
core computes only its own group's combined weight w_g[t]). Each core returns
w_g[t] * y_g[t, :]; the host sums the 8 partial outputs.

Per-core dataflow (matmul operands float32r: full PE rate, ~1.5e-4 rel err;
router logits in exact fp32 via bitcast so top-8 selection matches the fp32
reference on near-tie tokens):
  for each 128-token chunk:
    h[t,0:1536] = xT-chunk.T @ guT          (PE f32r, K=2048)
    logits[t,0:32] = xT-chunk.T @ gwT       (PE fp32, shares stationary)
    act[t,0:768] = silu(h[:,:768]) * h[:,768:]   (ACT silu, DVE mult)
    actT = PE-transpose(act)                 (PE + DVE psum->sbuf)
    y[t,:] = actT.T @ dnT                    (PE f32r, K=768)
    top-8(logits) -> w_g[t]                  (DVE chain + ACT exp)
    out[t,:] = w_g[t] * y[t,:] -> DRAM       (DVE tensor_scalar)
"""
import numpy as np

import concourse.bass as bass
import concourse.mybir as mybir
import concourse.tile as tile
from concourse import bacc
from concourse.bass_utils import run_bass_kernel_spmd
from concourse.masks import make_identity

T = 2048          # tokens
H = 2048          # hidden
I2 = 1536         # 2 * intermediate
I = 768           # intermediate
E = 32            # experts
G = 8             # groups / cores
TOP_K = 8
KO = H // 128     # 16 k-subtiles for the H contraction
JO = I // 128     # 6 k-subtiles for the I contraction
TCH = 128         # token chunk
NCHUNK = T // TCH # 16
HB = 512          # h/output column chunk
NEG_BIG = -1.0e9

F32 = mybir.dt.float32
F32R = mybir.dt.float32r
U8 = mybir.dt.uint8
AX = mybir.AxisListType.X
OP = mybir.AluOpType
ACTF = mybir.ActivationFunctionType

_CACHED_NC = None


def _build():
    global _CACHED_NC
    if _CACHED_NC is not None:
        return _CACHED_NC
    nc = bacc.Bacc("TRN2", target_bir_lowering=False, debug=False, num_devices=G)

    xT_d = nc.dram_tensor("xT", [H, T], F32R, kind="ExternalInput")
    gu_d = nc.dram_tensor("gu", [H, I2], F32R, kind="ExternalInput")
    gw_d = nc.dram_tensor("gw", [H, E], F32R, kind="ExternalInput")
    dnT_d = nc.dram_tensor("dnT", [I, H], F32R, kind="ExternalInput")
    mgb_d = nc.dram_tensor("mgb", [128, E], F32, kind="ExternalInput")
    y_d = nc.dram_tensor("y", [T, H], F32, kind="ExternalOutput")

    xT_ap = xT_d.ap().rearrange("(ko p) t -> p ko t", p=128)
    gu_ap = gu_d.ap().rearrange("(ko p) o -> p ko o", p=128)
    gw_ap = gw_d.ap().rearrange("(ko p) e -> p ko e", p=128)
    dnT_ap = dnT_d.ap().rearrange("(jo p) h -> p jo h", p=128)

    with tile.TileContext(nc) as tc:
        with (
            tc.tile_pool(name="const", bufs=1) as cpool,
            tc.tile_pool(name="weights", bufs=1) as wpool,
            tc.tile_pool(name="xin", bufs=3) as xpool,
            tc.tile_pool(name="acts", bufs=2) as apool,
            tc.tile_pool(name="router", bufs=2) as rpool,
            tc.tile_pool(name="yout", bufs=3) as ypool,
            tc.tile_pool(name="ph", bufs=1, space="PSUM") as pph,
            tc.tile_pool(name="ps", bufs=1, space="PSUM") as pps,
            tc.tile_pool(name="py", bufs=3, space="PSUM") as ppy,
        ):
            identity = cpool.tile([128, 128], F32, tag="identity")
            make_identity(nc, identity)
            negbig = cpool.tile([128, E], F32, tag="negbig")
            nc.vector.memset(negbig, NEG_BIG)
            mgb_sb = cpool.tile([128, E], F32, tag="mgb")
            nc.sync.dma_start(mgb_sb[:], mgb_d.ap())
            gw_sb = cpool.tile([128, KO, E], F32R, tag="gw")
            nc.sync.dma_start(gw_sb[:], gw_ap)

            # gate_up weights: split DMA per k-slice so matmuls start early
            gu_sb = wpool.tile([128, KO, I2], F32R, tag="gu")
            for k in range(KO):
                nc.sync.dma_start(gu_sb[:, k], gu_ap[:, k])

            xtiles = {}

            def load_chunk(ci):
                t = xpool.tile([128, KO, TCH], F32R, tag="xT_c",
                               name=f"xT_c{ci}")
                nc.sync.dma_start(t[:], xT_ap[:, :, ci * TCH:(ci + 1) * TCH])
                xtiles[ci] = t

            load_chunk(0)
            load_chunk(1)

            dn_sb = wpool.tile([128, JO, H], F32R, tag="dn")
            for j in range(JO):
                nc.sync.dma_start(dn_sb[:, j], dnT_ap[:, j])

            for tci in range(NCHUNK):
                tsl = slice(tci * TCH, (tci + 1) * TCH)
                if tci + 2 < NCHUNK:
                    load_chunk(tci + 2)
                xT_c = xtiles.pop(tci)

                # ---- M1: h (3x512 cols, f32r) + router logits (fp32) ----
                h_ps = pph.tile([128, 3, HB], F32, tag="h_ps")
                s_ps = pps.tile([128, 8, 128], F32, tag="s_ps")
                for k in range(KO):
                    st, sp = (k == 0), (k == KO - 1)
                    nc.tensor.matmul(
                        s_ps[:, 0, :E], xT_c[:, k].bitcast(F32),
                        gw_sb[:, k].bitcast(F32),
                        start=st, stop=sp,
                    )
                    for b in range(3):
                        nc.tensor.matmul(
                            h_ps[:, b], xT_c[:, k],
                            gu_sb[:, k, b * HB:(b + 1) * HB],
                            start=st, stop=sp,
                        )

                logits = rpool.tile([128, E], F32, tag="logits")
                nc.vector.tensor_copy(logits[:], s_ps[:, 0, :E])

                # ---- SwiGLU: act = silu(h[:, :768]) * h[:, 768:1536] ----
                silu_sb = apool.tile([128, I], F32, tag="silu")
                nc.scalar.activation(silu_sb[:, :HB], h_ps[:, 0], ACTF.Silu)
                nc.scalar.activation(silu_sb[:, HB:I], h_ps[:, 1, :I - HB], ACTF.Silu)
                act_sb = apool.tile([128, I], F32, tag="act")
                nc.vector.tensor_tensor(
                    act_sb[:, :I - HB], silu_sb[:, :I - HB],
                    h_ps[:, 1, I - HB:], OP.mult,
                )
                nc.vector.tensor_tensor(
                    act_sb[:, I - HB:], silu_sb[:, I - HB:],
                    h_ps[:, 2], OP.mult,
                )

                # ---- transpose act -> actT (PE), psum->sbuf on DVE ----
                actT_sb = apool.tile([128, JO, TCH], F32R, tag="actT")
                for j in range(JO):
                    nc.tensor.transpose(
                        s_ps[:, 2 + j], act_sb[:, j * 128:(j + 1) * 128], identity,
                    )
                    nc.vector.tensor_copy(actT_sb[:, j], s_ps[:, 2 + j])

                # ---- M2: y = actT.T @ dnT ----
                y_pss = []
                for hb in range(H // HB):
                    y_ps = ppy.tile([128, HB], F32, tag="y_ps",
                                    name=f"y_ps{tci}_{hb}")
                    for j in range(JO):
                        nc.tensor.matmul(
                            y_ps[:], actT_sb[:, j],
                            dn_sb[:, j, hb * HB:(hb + 1) * HB],
                            start=(j == 0), stop=(j == JO - 1),
                        )
                    y_pss.append(y_ps)

                # ---- router: top-8 -> per-token group weight (DVE) ----
                cur = rpool.tile([128, E], F32, tag="cur")
                nc.vector.tensor_copy(cur[:], logits[:])
                msk = rpool.tile([128, E], U8, tag="msk")
                m1 = rpool.tile([128, 1], F32, tag="m1")
                mk = rpool.tile([128, 1], F32, tag="mk")
                for it in range(TOP_K - 1):
                    tgt = m1 if it == 0 else mk
                    nc.vector.reduce_max(tgt[:], cur[:], axis=AX)
                    nc.vector.tensor_scalar(msk[:], cur[:], tgt[:], None, OP.is_ge)
                    nc.vector.copy_predicated(cur[:], msk[:], negbig[:])
                m8 = rpool.tile([128, 1], F32, tag="m8")
                nc.vector.reduce_max(m8[:], cur[:], axis=AX)

                nm1 = rpool.tile([128, 1], F32, tag="nm1")
                nc.vector.tensor_scalar(nm1[:], m1[:], -1.0, None, OP.mult)
                mask8 = rpool.tile([128, E], F32, tag="mask8")
                nc.vector.tensor_scalar(mask8[:], logits[:], m8[:], None, OP.is_ge)
                ew = rpool.tile([128, E], F32, tag="ew")
                nc.scalar.activation(ew[:], logits[:], ACTF.Exp, bias=nm1[:])
                nc.vector.tensor_tensor(ew[:], ew[:], mask8[:], OP.mult)
                s8 = rpool.tile([128, 1], F32, tag="s8")
                nc.vector.reduce_sum(s8[:], ew[:], axis=AX)
                nc.vector.tensor_tensor(ew[:], ew[:], mgb_sb[:], OP.mult)
                num = rpool.tile([128, 1], F32, tag="num")
                nc.vector.reduce_sum(num[:], ew[:], axis=AX)
                rs = rpool.tile([128, 1], F32, tag="rs")
                nc.vector.reciprocal(rs[:], s8[:])
                w_t = rpool.tile([128, 1], F32, tag="w_t")
                nc.vector.tensor_tensor(w_t[:], num[:], rs[:], OP.mult)

                # ---- scale + store ----
                for hb in range(H // HB):
                    y_sb = ypool.tile([128, HB], F32, tag="y_sb")
                    nc.vector.tensor_scalar(
                        y_sb[:], y_pss[hb][:], w_t[:], None, OP.mult,
                    )
                    nc.sync.dma_start(
                        y_d.ap()[tsl, hb * HB:(hb + 1) * HB], y_sb[:],
                    )
    nc.compile()
    _CACHED_NC = nc
    return nc


def prepare_in_maps(hidden_states, gate_weight, gate_up_proj, down_proj,
                    merge_groups, dominant_experts):
    x = np.asarray(hidden_states, dtype=np.float32).reshape(T, H)
    xT = np.ascontiguousarray(x.T)
    gw = np.asarray(gate_weight, dtype=np.float32)
    gwT = np.ascontiguousarray(gw.T)  # [H, E]
    mg = np.asarray(merge_groups).astype(np.int64)
    de = np.asarray(dominant_experts).astype(np.int64)
    gup = np.asarray(gate_up_proj, dtype=np.float32)
    dnp_ = np.asarray(down_proj, dtype=np.float32)

    in_maps = []
    for g in range(G):
        e = int(de[g])
        guT = np.ascontiguousarray(gup[e].T)   # [H, 1536]
        dnT = np.ascontiguousarray(dnp_[e].T)  # [I, H]
        mgb = np.ascontiguousarray(
            np.broadcast_to((mg == g).astype(np.float32)[None, :], (128, E))
        )
        in_maps.append({"xT": xT, "gu": guT, "gw": gwT, "dnT": dnT, "mgb": mgb})
    return in_maps


def kernel(hidden_states, gate_weight, gate_up_proj, down_proj,
           merge_groups, dominant_experts):
    in_maps = prepare_in_maps(hidden_states, gate_weight, gate_up_proj,
                              down_proj, merge_groups, dominant_experts)
    nc = _build()
    res = run_bass_kernel_spmd(nc, in_maps, core_ids=list(range(G)), trace=False)
    out = np.zeros((T, H), dtype=np.float64)
    for r in res.results:
        out += r["y"].astype(np.float64)
    return out.astype(np.float32).reshape(1, T, H)


# revision 8
# speedup vs baseline: 1.1801x; 1.1801x over previous
"""TRN2 Bass kernel for nn_HCSMoEQwen3MoeSparseMoeBlock (8-core expert-parallel).

Sharding: core g owns group g's dominant expert and processes ALL tokens;
router replicated (each core computes only its group's combined weight
w_g[t]); host sums the 8 partial outputs w_g[t] * y_g[t, :].

Structure (per core):
  Router pre-pass (overlaps the weight DMA window): exact-fp32 logits via
  gw-stationary matmuls, logitsT -> PE transpose -> logits_all [T,32].
  Main loop per 128-token chunk, float32r matmuls (full PE rate, ~2e-4):
    b-major M1: for each 512-col block b (host-interleaved [256 gate|256 up]):
      h_b = xT_c.T @ gu_b (16 same-bank MMs), silu+mult drain bank early
    actT = PE-transpose(act);  y = actT.T @ dnT (per 512-col out chunk)
    top-8 chain on DVE (pinned after casts);  out = w*y -> DRAM
"""
import numpy as np

import concourse.bass as bass
import concourse.mybir as mybir
import concourse.tile as tile
from concourse import bacc
from concourse.bass_utils import run_bass_kernel_spmd
from concourse.masks import make_identity

T = 2048
H = 2048
I2 = 1536
I = 768
E = 32
G = 8
TOP_K = 8
KO = H // 128
JO = I // 128
TCH = 128
NCHUNK = T // TCH
HB = 512
NEG_BIG = -1.0e9

F32 = mybir.dt.float32
F32R = mybir.dt.float32r
U8 = mybir.dt.uint8
AX = mybir.AxisListType.X
OP = mybir.AluOpType
ACTF = mybir.ActivationFunctionType

_CACHED_NC = None


def _build():
    global _CACHED_NC
    if _CACHED_NC is not None:
        return _CACHED_NC
    nc = bacc.Bacc("TRN2", target_bir_lowering=False, debug=False, num_devices=G)

    xT_d = nc.dram_tensor("xT", [H, T], F32R, kind="ExternalInput")
    gu_d = nc.dram_tensor("gu", [H, I2], F32R, kind="ExternalInput")
    gw_d = nc.dram_tensor("gw", [H, E], F32, kind="ExternalInput")
    dnT_d = nc.dram_tensor("dnT", [I, H], F32R, kind="ExternalInput")
    mgb_d = nc.dram_tensor("mgb", [128, E], F32, kind="ExternalInput")
    y_d = nc.dram_tensor("y", [T, H], F32, kind="ExternalOutput")

    xT_ap = xT_d.ap().rearrange("(ko p) t -> p ko t", p=128)
    xT_ap32 = xT_d.ap().bitcast(F32).rearrange("(ko p) t -> p ko t", p=128)
    gu_ap = gu_d.ap().rearrange("(ko p) o -> p ko o", p=128)
    gw_ap = gw_d.ap().rearrange("(ko p) e -> p ko e", p=128)
    dnT_ap = dnT_d.ap().rearrange("(jo p) h -> p jo h", p=128)

    with tile.TileContext(nc) as tc:
        with (
            tc.tile_pool(name="const", bufs=1) as cpool,
            tc.tile_pool(name="weights", bufs=1) as wpool,
            tc.tile_pool(name="xin", bufs=3) as xpool,
            tc.tile_pool(name="router", bufs=2) as rpool,
        ):
            identity = cpool.tile([128, 128], F32, tag="identity")
            make_identity(nc, identity)
            negbig = cpool.tile([128, E], F32, tag="negbig")
            nc.vector.memset(negbig, NEG_BIG)
            mgb_sb = cpool.tile([128, E], F32, tag="mgb")
            nc.sync.dma_start(mgb_sb[:], mgb_d.ap())
            gw_sb = cpool.tile([128, KO, E], F32, tag="gw")
            nc.sync.dma_start(gw_sb[:], gw_ap)
            logits_all = cpool.tile([128, NCHUNK, E], F32, tag="logits_all")

            gu_sb = wpool.tile([128, KO, I2], F32R, tag="gu")
            dn_sb = wpool.tile([128, JO, H], F32R, tag="dn")

            xtiles = {}

            def load_chunk(ci):
                t = xpool.tile([128, KO, TCH], F32R, tag="xT_c",
                               name=f"xT_c{ci}")
                nc.sync.dma_start(t[:], xT_ap[:, :, ci * TCH:(ci + 1) * TCH])
                xtiles[ci] = t

            # ---- router pre-pass: exact fp32 logits for all chunks ----
            # (interleave weight-DMA emissions so they stream during it)
            with (
                tc.tile_pool(name="prt", bufs=2, space="PSUM") as pprt,
                tc.tile_pool(name="xrin", bufs=2) as xrpool,
            ):
                for rc in range(NCHUNK):
                    xr = xrpool.tile([128, KO, TCH], F32, tag="xr",
                                     name=f"xr{rc}")
                    nc.sync.dma_start(
                        xr[:], xT_ap32[:, :, rc * TCH:(rc + 1) * TCH])
                    lgT = pprt.tile([E, TCH], F32, tag="lgT")
                    for k in range(KO):
                        nc.tensor.matmul(
                            lgT[:], gw_sb[:, k], xr[:, k],
                            start=(k == 0), stop=(k == KO - 1),
                        )
                    lgT_sb = rpool.tile([E, TCH], F32, tag="lgT_sb")
                    nc.vector.tensor_copy(lgT_sb[:], lgT[:])
                    tp = pprt.tile([128, E], F32, tag="tp")
                    nc.tensor.transpose(tp[:], lgT_sb[:], identity[:E, :E])
                    nc.vector.tensor_copy(logits_all[:, rc], tp[:])

                    # stream the big weights behind the router traffic
                    if rc == 1:
                        nc.sync.dma_start(gu_sb[:, :, 0:HB],
                                          gu_ap[:, :, 0:HB])
                    elif rc == 4:
                        nc.sync.dma_start(gu_sb[:, :, HB:2 * HB],
                                          gu_ap[:, :, HB:2 * HB])
                    elif rc == 7:
                        nc.sync.dma_start(gu_sb[:, :, 2 * HB:I2],
                                          gu_ap[:, :, 2 * HB:I2])
                    elif rc == 9:
                        load_chunk(0)
                    elif rc == 11:
                        load_chunk(1)
                    elif rc == 13:
                        for j in range(JO):
                            nc.sync.dma_start(dn_sb[:, j], dnT_ap[:, j])

            with (
                tc.tile_pool(name="acts", bufs=2) as apool,
                tc.tile_pool(name="yout", bufs=2) as ypool,
                tc.tile_pool(name="ph", bufs=3, space="PSUM") as pph,
                tc.tile_pool(name="ps", bufs=1, space="PSUM") as pps,
                tc.tile_pool(name="py", bufs=3, space="PSUM") as ppy,
            ):
                for tci in range(NCHUNK):
                    tsl = slice(tci * TCH, (tci + 1) * TCH)
                    if tci + 2 < NCHUNK:
                        load_chunk(tci + 2)
                    xT_c = xtiles.pop(tci)

                    # ---- M1, b-major: one PSUM bank at a time ----
                    act_sb = apool.tile([128, I], F32, tag="act")
                    for b in range(3):
                        h_ps = pph.tile([128, HB], F32, tag="h_ps",
                                        name=f"h{tci}_{b}")
                        for k in range(KO):
                            nc.tensor.matmul(
                                h_ps[:], xT_c[:, k],
                                gu_sb[:, k, b * HB:(b + 1) * HB],
                                start=(k == 0), stop=(k == KO - 1),
                            )
                        # host interleave: h_b = [256 gate | 256 up]
                        silu_sb = apool.tile([128, 256], F32, tag="silu")
                        nc.scalar.activation(silu_sb[:], h_ps[:, :256],
                                             ACTF.Silu)
                        nc.vector.tensor_tensor(
                            act_sb[:, 256 * b:256 * (b + 1)], silu_sb[:],
                            h_ps[:, 256:], OP.mult,
                        )

                    # ---- transpose act -> actT ----
                    s_ps = pps.tile([128, JO, TCH], F32, tag="s_ps")
                    actT_sb = apool.tile([128, JO, TCH], F32R, tag="actT")
                    for j in range(JO):
                        nc.tensor.transpose(
                            s_ps[:, j], act_sb[:, j * 128:(j + 1) * 128],
                            identity,
                        )
                        nc.vector.tensor_copy(actT_sb[:, j], s_ps[:, j])

                    # ---- M2 ----
                    y_pss = []
                    for hb in range(H // HB):
                        y_ps = ppy.tile([128, HB], F32, tag="y_ps",
                                        name=f"y_ps{tci}_{hb}")
                        for j in range(JO):
                            nc.tensor.matmul(
                                y_ps[:], actT_sb[:, j],
                                dn_sb[:, j, hb * HB:(hb + 1) * HB],
                                start=(j == 0), stop=(j == JO - 1),
                            )
                        y_pss.append(y_ps)

                    # ---- top-8 router chain (DVE), pinned after casts ----
                    dep = rpool.tile([128, E], F32, tag="dep")
                    nc.vector.tensor_scalar(
                        dep[:], actT_sb[:, JO - 1, :E].bitcast(F32), 0.0,
                        None, OP.mult)
                    cur = rpool.tile([128, E], F32, tag="cur")
                    nc.vector.tensor_tensor(cur[:], logits_all[:, tci],
                                            dep[:], OP.add)
                    msk = rpool.tile([128, E], U8, tag="msk")
                    m1 = rpool.tile([128, 1], F32, tag="m1")
                    mk = rpool.tile([128, 1], F32, tag="mk")
                    for it in range(TOP_K - 1):
                        tgt = m1 if it == 0 else mk
                        nc.vector.reduce_max(tgt[:], cur[:], axis=AX)
                        nc.vector.tensor_scalar(msk[:], cur[:], tgt[:],
                                                None, OP.is_ge)
                        nc.vector.copy_predicated(cur[:], msk[:], negbig[:])
                    m8 = rpool.tile([128, 1], F32, tag="m8")
                    nc.vector.reduce_max(m8[:], cur[:], axis=AX)

                    nm1 = rpool.tile([128, 1], F32, tag="nm1")
                    nc.vector.tensor_scalar(nm1[:], m1[:], -1.0, None, OP.mult)
                    mask8 = rpool.tile([128, E], F32, tag="mask8")
                    nc.vector.tensor_scalar(mask8[:], logits_all[:, tci],
                                            m8[:], None, OP.is_ge)
                    ew = rpool.tile([128, E], F32, tag="ew")
                    nc.scalar.activation(ew[:], logits_all[:, tci], ACTF.Exp,
                                         bias=nm1[:])
                    nc.vector.tensor_tensor(ew[:], ew[:], mask8[:], OP.mult)
                    s8 = rpool.tile([128, 1], F32, tag="s8")
                    nc.vector.reduce_sum(s8[:], ew[:], axis=AX)
                    nc.vector.tensor_tensor(ew[:], ew[:], mgb_sb[:], OP.mult)
                    num = rpool.tile([128, 1], F32, tag="num")
                    nc.vector.reduce_sum(num[:], ew[:], axis=AX)
                    rs = rpool.tile([128, 1], F32, tag="rs")
                    nc.vector.reciprocal(rs[:], s8[:])
                    w_t = rpool.tile([128, 1], F32, tag="w_t")
                    nc.vector.tensor_tensor(w_t[:], num[:], rs[:], OP.mult)

                    # ---- scale + store ----
                    for hb in range(H // HB):
                        y_sb = ypool.tile([128, HB], F32, tag="y_sb")
                        nc.vector.tensor_scalar(
                            y_sb[:], y_pss[hb][:], w_t[:], None, OP.mult,
                        )
                        nc.sync.dma_start(
                            y_d.ap()[tsl, hb * HB:(hb + 1) * HB], y_sb[:],
                        )
    nc.compile()
    _CACHED_NC = nc
    return nc


_GATEUP_PERM = np.concatenate(
    [np.r_[256 * b:256 * b + 256, 768 + 256 * b:768 + 256 * b + 256]
     for b in range(3)]
)


def prepare_in_maps(hidden_states, gate_weight, gate_up_proj, down_proj,
                    merge_groups, dominant_experts):
    x = np.asarray(hidden_states, dtype=np.float32).reshape(T, H)
    xT = np.ascontiguousarray(x.T)
    gw = np.asarray(gate_weight, dtype=np.float32)
    gwT = np.ascontiguousarray(gw.T)
    mg = np.asarray(merge_groups).astype(np.int64)
    de = np.asarray(dominant_experts).astype(np.int64)
    gup = np.asarray(gate_up_proj, dtype=np.float32)
    dnp_ = np.asarray(down_proj, dtype=np.float32)

    in_maps = []
    for g in range(G):
        e = int(de[g])
        guT = np.ascontiguousarray(gup[e].T[:, _GATEUP_PERM])
        dnT = np.ascontiguousarray(dnp_[e].T)
        mgb = np.ascontiguousarray(
            np.broadcast_to((mg == g).astype(np.float32)[None, :], (128, E))
        )
        in_maps.append({"xT": xT, "gu": guT, "gw": gwT, "dnT": dnT,
                        "mgb": mgb})
    return in_maps


def kernel(hidden_states, gate_weight, gate_up_proj, down_proj,
           merge_groups, dominant_experts):
    in_maps = prepare_in_maps(hidden_states, gate_weight, gate_up_proj,
                              down_proj, merge_groups, dominant_experts)
    nc = _build()
    res = run_bass_kernel_spmd(nc, in_maps, core_ids=list(range(G)),
                               trace=False)
    out = np.zeros((T, H), dtype=np.float64)
    for r in res.results:
        out += r["y"].astype(np.float64)
    return out.astype(np.float32).reshape(1, T, H)


# revision 9
# speedup vs baseline: 1.2019x; 1.0184x over previous
"""TRN2 Bass kernel for nn_HCSMoEQwen3MoeSparseMoeBlock (8-core expert-parallel).

Sharding: core g owns group g's dominant expert and processes ALL tokens;
router replicated (each core computes only its group's combined weight
w_g[t]); host sums the 8 partial outputs w_g[t] * y_g[t, :].

Single software-pipelined loop over 128-token chunks; float32r matmuls
(full PE rate, ~2e-4 rel err); router logits in exact fp32 (separate
F32-typed tiles — the PE precision mode follows the backing tensor dtype):
  router: logitsT = gwT.T-stationary @ x-chunk (fp32) -> PE transpose
  M1 b-major: h_b = xT_c.T @ gu_b, 16 same-bank MMs per 512-col block
              (host-interleaved [256 gate|256 up]) -> silu+mult drains bank
  actT = PE-transpose(act);  y = actT.T @ dnT;  top-8 chain on DVE
  (pinned after casts);  out = w*y -> DRAM
"""
import numpy as np

import concourse.bass as bass
import concourse.mybir as mybir
import concourse.tile as tile
from concourse import bacc
from concourse.bass_utils import run_bass_kernel_spmd
from concourse.masks import make_identity

T = 2048
H = 2048
I2 = 1536
I = 768
E = 32
G = 8
TOP_K = 8
KO = H // 128
JO = I // 128
TCH = 128
NCHUNK = T // TCH
HB = 512
NEG_BIG = -1.0e9

F32 = mybir.dt.float32
F32R = mybir.dt.float32r
U8 = mybir.dt.uint8
AX = mybir.AxisListType.X
OP = mybir.AluOpType
ACTF = mybir.ActivationFunctionType

_CACHED_NC = None


def _build():
    global _CACHED_NC
    if _CACHED_NC is not None:
        return _CACHED_NC
    nc = bacc.Bacc("TRN2", target_bir_lowering=False, debug=False, num_devices=G)

    xT_d = nc.dram_tensor("xT", [H, T], F32R, kind="ExternalInput")
    gu_d = nc.dram_tensor("gu", [H, I2], F32R, kind="ExternalInput")
    gw_d = nc.dram_tensor("gw", [H, E], F32, kind="ExternalInput")
    dnT_d = nc.dram_tensor("dnT", [I, H], F32R, kind="ExternalInput")
    mgb_d = nc.dram_tensor("mgb", [128, E], F32, kind="ExternalInput")
    y_d = nc.dram_tensor("y", [T, H], F32, kind="ExternalOutput")

    xT_ap = xT_d.ap().rearrange("(ko p) t -> p ko t", p=128)
    xT_ap32 = xT_d.ap().bitcast(F32).rearrange("(ko p) t -> p ko t", p=128)
    gu_ap = gu_d.ap().rearrange("(ko p) o -> p ko o", p=128)
    gw_ap = gw_d.ap().rearrange("(ko p) e -> p ko e", p=128)
    dnT_ap = dnT_d.ap().rearrange("(jo p) h -> p jo h", p=128)

    with tile.TileContext(nc) as tc:
        with (
            tc.tile_pool(name="const", bufs=1) as cpool,
            tc.tile_pool(name="weights", bufs=1) as wpool,
            tc.tile_pool(name="xin", bufs=2) as xpool,
            tc.tile_pool(name="xrin", bufs=2) as xrpool,
            tc.tile_pool(name="acts", bufs=1) as apool,
            tc.tile_pool(name="router", bufs=2) as rpool,
            tc.tile_pool(name="yout", bufs=2) as ypool,
            tc.tile_pool(name="plg", bufs=1, space="PSUM") as plg,
            tc.tile_pool(name="ph", bufs=3, space="PSUM") as pph,
            tc.tile_pool(name="ps", bufs=1, space="PSUM") as pps,
            tc.tile_pool(name="py", bufs=2, space="PSUM") as ppy,
        ):
            identity = cpool.tile([128, 128], F32, tag="identity")
            make_identity(nc, identity)
            negbig = cpool.tile([128, E], F32, tag="negbig")
            nc.vector.memset(negbig, NEG_BIG)
            mgb_sb = cpool.tile([128, E], F32, tag="mgb")
            nc.sync.dma_start(mgb_sb[:], mgb_d.ap())
            gw_sb = cpool.tile([128, KO, E], F32, tag="gw")
            nc.sync.dma_start(gw_sb[:], gw_ap)

            gu_sb = wpool.tile([128, KO, I2], F32R, tag="gu")
            dn_sb = wpool.tile([128, JO, H], F32R, tag="dn")

            xtiles = {}
            xrtiles = {}

            def load_chunk(ci):
                t = xpool.tile([128, KO, TCH], F32R, tag="xT_c",
                               name=f"xT_c{ci}")
                nc.sync.dma_start(t[:], xT_ap[:, :, ci * TCH:(ci + 1) * TCH])
                xtiles[ci] = t

            def load_xr(ci):
                t = xrpool.tile([128, KO, TCH], F32, tag="xr",
                                name=f"xr{ci}")
                nc.sync.dma_start(t[:], xT_ap32[:, :, ci * TCH:(ci + 1) * TCH])
                xrtiles[ci] = t

            load_xr(0)
            load_chunk(0)
            nc.sync.dma_start(gu_sb[:, :, 0:HB], gu_ap[:, :, 0:HB])

            for tci in range(NCHUNK):
                tsl = slice(tci * TCH, (tci + 1) * TCH)
                if tci + 1 < NCHUNK:
                    load_xr(tci + 1)
                    load_chunk(tci + 1)
                xT_c = xtiles.pop(tci)
                xr_c = xrtiles.pop(tci)

                # ---- router logits (exact fp32), gw stationary ----
                lg_ps = plg.tile([E, TCH], F32, tag="lg_ps")
                for k in range(KO):
                    nc.tensor.matmul(
                        lg_ps[:], gw_sb[:, k], xr_c[:, k],
                        start=(k == 0), stop=(k == KO - 1),
                    )
                lgT_sb = rpool.tile([E, TCH], F32, tag="lgT_sb")
                nc.vector.tensor_copy(lgT_sb[:], lg_ps[:])
                s_ps = pps.tile([128, JO + 1, TCH], F32, tag="s_ps")
                nc.tensor.transpose(s_ps[:, JO, :E], lgT_sb[:],
                                    identity[:E, :E])
                logits = rpool.tile([128, E], F32, tag="logits")
                nc.vector.tensor_copy(logits[:], s_ps[:, JO, :E])

                if tci == 0:
                    nc.sync.dma_start(gu_sb[:, :, HB:2 * HB],
                                      gu_ap[:, :, HB:2 * HB])

                # ---- M1, b-major: one PSUM bank at a time ----
                act_sb = apool.tile([128, I], F32, tag="act")
                for b in range(3):
                    h_ps = pph.tile([128, HB], F32, tag="h_ps",
                                    name=f"h{tci}_{b}")
                    for k in range(KO):
                        nc.tensor.matmul(
                            h_ps[:], xT_c[:, k],
                            gu_sb[:, k, b * HB:(b + 1) * HB],
                            start=(k == 0), stop=(k == KO - 1),
                        )
                    if tci == 0 and b == 0:
                        nc.sync.dma_start(gu_sb[:, :, 2 * HB:I2],
                                          gu_ap[:, :, 2 * HB:I2])
                    if tci == 0 and b == 1:
                        for j in range(JO):
                            nc.sync.dma_start(dn_sb[:, j], dnT_ap[:, j])
                    # host interleave: h_b = [256 gate | 256 up]
                    silu_sb = apool.tile([128, 256], F32, tag="silu")
                    nc.scalar.activation(silu_sb[:], h_ps[:, :256],
                                         ACTF.Silu)
                    nc.vector.tensor_tensor(
                        act_sb[:, 256 * b:256 * (b + 1)], silu_sb[:],
                        h_ps[:, 256:], OP.mult,
                    )

                # ---- transpose act -> actT ----
                actT_sb = apool.tile([128, JO, TCH], F32R, tag="actT")
                for j in range(JO):
                    nc.tensor.transpose(
                        s_ps[:, j], act_sb[:, j * 128:(j + 1) * 128],
                        identity,
                    )
                    nc.vector.tensor_copy(actT_sb[:, j], s_ps[:, j])

                # ---- M2 ----
                y_pss = []
                for hb in range(H // HB):
                    y_ps = ppy.tile([128, HB], F32, tag="y_ps",
                                    name=f"y_ps{tci}_{hb}")
                    for j in range(JO):
                        nc.tensor.matmul(
                            y_ps[:], actT_sb[:, j],
                            dn_sb[:, j, hb * HB:(hb + 1) * HB],
                            start=(j == 0), stop=(j == JO - 1),
                        )
                    y_pss.append(y_ps)

                # ---- top-8 router chain (DVE), pinned after casts ----
                dep = rpool.tile([128, E], F32, tag="dep")
                nc.vector.tensor_scalar(
                    dep[:], actT_sb[:, JO - 1, :E].bitcast(F32), 0.0,
                    None, OP.mult)
                cur = rpool.tile([128, E], F32, tag="cur")
                nc.vector.tensor_tensor(cur[:], logits[:], dep[:], OP.add)
                msk = rpool.tile([128, E], U8, tag="msk")
                m1 = rpool.tile([128, 1], F32, tag="m1")
                mk = rpool.tile([128, 1], F32, tag="mk")
                for it in range(TOP_K - 1):
                    tgt = m1 if it == 0 else mk
                    nc.vector.reduce_max(tgt[:], cur[:], axis=AX)
                    nc.vector.tensor_scalar(msk[:], cur[:], tgt[:],
                                            None, OP.is_ge)
                    nc.vector.copy_predicated(cur[:], msk[:], negbig[:])
                m8 = rpool.tile([128, 1], F32, tag="m8")
                nc.vector.reduce_max(m8[:], cur[:], axis=AX)

                nm1 = rpool.tile([128, 1], F32, tag="nm1")
                nc.vector.tensor_scalar(nm1[:], m1[:], -1.0, None, OP.mult)
                mask8 = rpool.tile([128, E], F32, tag="mask8")
                nc.vector.tensor_scalar(mask8[:], logits[:], m8[:],
                                        None, OP.is_ge)
                ew = rpool.tile([128, E], F32, tag="ew")
                nc.scalar.activation(ew[:], logits[:], ACTF.Exp, bias=nm1[:])
                nc.vector.tensor_tensor(ew[:], ew[:], mask8[:], OP.mult)
                s8 = rpool.tile([128, 1], F32, tag="s8")
                nc.vector.reduce_sum(s8[:], ew[:], axis=AX)
                nc.vector.tensor_tensor(ew[:], ew[:], mgb_sb[:], OP.mult)
                num = rpool.tile([128, 1], F32, tag="num")
                nc.vector.reduce_sum(num[:], ew[:], axis=AX)
                rs = rpool.tile([128, 1], F32, tag="rs")
                nc.vector.reciprocal(rs[:], s8[:])
                w_t = rpool.tile([128, 1], F32, tag="w_t")
                nc.vector.tensor_tensor(w_t[:], num[:], rs[:], OP.mult)

                # ---- scale + store ----
                for hb in range(H // HB):
                    y_sb = ypool.tile([128, HB], F32, tag="y_sb")
                    nc.vector.tensor_scalar(
                        y_sb[:], y_pss[hb][:], w_t[:], None, OP.mult,
                    )
                    nc.sync.dma_start(
                        y_d.ap()[tsl, hb * HB:(hb + 1) * HB], y_sb[:],
                    )
    nc.compile()
    _CACHED_NC = nc
    return nc


_GATEUP_PERM = np.concatenate(
    [np.r_[256 * b:256 * b + 256, 768 + 256 * b:768 + 256 * b + 256]
     for b in range(3)]
)


def prepare_in_maps(hidden_states, gate_weight, gate_up_proj, down_proj,
                    merge_groups, dominant_experts):
    x = np.asarray(hidden_states, dtype=np.float32).reshape(T, H)
    xT = np.ascontiguousarray(x.T)
    gw = np.asarray(gate_weight, dtype=np.float32)
    gwT = np.ascontiguousarray(gw.T)
    mg = np.asarray(merge_groups).astype(np.int64)
    de = np.asarray(dominant_experts).astype(np.int64)
    gup = np.asarray(gate_up_proj, dtype=np.float32)
    dnp_ = np.asarray(down_proj, dtype=np.float32)

    in_maps = []
    for g in range(G):
        e = int(de[g])
        guT = np.ascontiguousarray(gup[e].T[:, _GATEUP_PERM])
        dnT = np.ascontiguousarray(dnp_[e].T)
        mgb = np.ascontiguousarray(
            np.broadcast_to((mg == g).astype(np.float32)[None, :], (128, E))
        )
        in_maps.append({"xT": xT, "gu": guT, "gw": gwT, "dnT": dnT,
                        "mgb": mgb})
    return in_maps


def kernel(hidden_states, gate_weight, gate_up_proj, down_proj,
           merge_groups, dominant_experts):
    in_maps = prepare_in_maps(hidden_states, gate_weight, gate_up_proj,
                              down_proj, merge_groups, dominant_experts)
    nc = _build()
    res = run_bass_kernel_spmd(nc, in_maps, core_ids=list(range(G)),
                               trace=False)
    out = np.zeros((T, H), dtype=np.float64)
    for r in res.results:
        out += r["y"].astype(np.float64)
    return out.astype(np.float32).reshape(1, T, H)


# revision 10
# speedup vs baseline: 1.2479x; 1.0383x over previous
"""TRN2 Bass kernel for nn_HCSMoEQwen3MoeSparseMoeBlock (8-core expert-parallel).

Sharding: core g owns group g's dominant expert and processes ALL tokens;
router replicated (each core computes only its group's combined weight
w_g[t]); host sums the 8 partial outputs w_g[t] * y_g[t, :].

Single software-pipelined loop over 128-token chunks; float32r matmuls
(full PE rate, ~2e-4 rel err); router logits in exact fp32 (separate
F32-typed tiles — the PE precision mode follows the backing tensor dtype):
  router: logitsT = gwT.T-stationary @ x-chunk (fp32) -> PE transpose
  M1 b-major: h_b = xT_c.T @ gu_b, 16 same-bank MMs per 512-col block
              (host-interleaved [256 gate|256 up]) -> silu+mult drains bank
  actT = PE-transpose(act);  y = actT.T @ dnT;  top-8 chain on DVE
  (pinned after casts);  out = w*y -> DRAM
"""
import numpy as np

import concourse.bass as bass
import concourse.mybir as mybir
import concourse.tile as tile
from concourse import bacc
from concourse.bass_utils import run_bass_kernel_spmd
from concourse.masks import make_identity

T = 2048
H = 2048
I2 = 1536
I = 768
E = 32
G = 8
TOP_K = 8
KO = H // 128
JO = I // 128
TCH = 128
NCHUNK = T // TCH
HB = 512
NEG_BIG = -1.0e9

F32 = mybir.dt.float32
F32R = mybir.dt.float32r
U8 = mybir.dt.uint8
AX = mybir.AxisListType.X
OP = mybir.AluOpType
ACTF = mybir.ActivationFunctionType

_CACHED_NC = None


def _build():
    global _CACHED_NC
    if _CACHED_NC is not None:
        return _CACHED_NC
    nc = bacc.Bacc("TRN2", target_bir_lowering=False, debug=False, num_devices=G)

    xT_d = nc.dram_tensor("xT", [H, T], F32R, kind="ExternalInput")
    gu_d = nc.dram_tensor("gu", [H, I2], F32R, kind="ExternalInput")
    gw_d = nc.dram_tensor("gw", [H, E], F32, kind="ExternalInput")
    dnT_d = nc.dram_tensor("dnT", [I, H], F32R, kind="ExternalInput")
    mgb_d = nc.dram_tensor("mgb", [128, E], F32, kind="ExternalInput")
    y_d = nc.dram_tensor("y", [T, H], F32, kind="ExternalOutput")

    xT_ap = xT_d.ap().rearrange("(ko p) t -> p ko t", p=128)
    xT_ap32 = xT_d.ap().bitcast(F32).rearrange("(ko p) t -> p ko t", p=128)
    gu_ap = gu_d.ap().rearrange("(ko p) o -> p ko o", p=128)
    gw_ap = gw_d.ap().rearrange("(ko p) e -> p ko e", p=128)
    dnT_ap = dnT_d.ap().rearrange("(jo p) h -> p jo h", p=128)

    with tile.TileContext(nc) as tc:
        with (
            tc.tile_pool(name="const", bufs=1) as cpool,
            tc.tile_pool(name="weights", bufs=1) as wpool,
            tc.tile_pool(name="xin", bufs=2) as xpool,
            tc.tile_pool(name="xrin", bufs=2) as xrpool,
            tc.tile_pool(name="acts", bufs=1) as apool,
            tc.tile_pool(name="router", bufs=2) as rpool,
            tc.tile_pool(name="yout", bufs=2) as ypool,
            tc.tile_pool(name="plg", bufs=1, space="PSUM") as plg,
            tc.tile_pool(name="ph", bufs=3, space="PSUM") as pph,
            tc.tile_pool(name="ps", bufs=1, space="PSUM") as pps,
            tc.tile_pool(name="py", bufs=2, space="PSUM") as ppy,
        ):
            identity = cpool.tile([128, 128], F32, tag="identity")
            make_identity(nc, identity)
            negbig = cpool.tile([128, E], F32, tag="negbig")
            nc.vector.memset(negbig, NEG_BIG)
            mgb_sb = cpool.tile([128, E], F32, tag="mgb")
            nc.sync.dma_start(mgb_sb[:], mgb_d.ap())
            gw_sb = cpool.tile([128, KO, E], F32, tag="gw")
            nc.sync.dma_start(gw_sb[:], gw_ap)

            gu_sb = wpool.tile([128, KO, I2], F32R, tag="gu")
            dn_sb = wpool.tile([128, JO, H], F32R, tag="dn")

            xtiles = {}
            xrtiles = {}

            def load_chunk(ci):
                t = xpool.tile([128, KO, TCH], F32R, tag="xT_c",
                               name=f"xT_c{ci}")
                nc.sync.dma_start(t[:], xT_ap[:, :, ci * TCH:(ci + 1) * TCH])
                xtiles[ci] = t

            def load_xr(ci):
                t = xrpool.tile([128, KO, TCH], F32, tag="xr",
                                name=f"xr{ci}")
                nc.sync.dma_start(t[:], xT_ap32[:, :, ci * TCH:(ci + 1) * TCH])
                xrtiles[ci] = t

            load_xr(0)
            load_chunk(0)
            nc.sync.dma_start(gu_sb[:, :, 0:HB], gu_ap[:, :, 0:HB])

            for tci in range(NCHUNK):
                tsl = slice(tci * TCH, (tci + 1) * TCH)
                if tci + 1 < NCHUNK:
                    load_xr(tci + 1)
                    load_chunk(tci + 1)
                xT_c = xtiles.pop(tci)
                xr_c = xrtiles.pop(tci)

                # ---- router logits (exact fp32), gw stationary ----
                lg_ps = plg.tile([E, TCH], F32, tag="lg_ps")
                for k in range(KO):
                    nc.tensor.matmul(
                        lg_ps[:], gw_sb[:, k], xr_c[:, k],
                        start=(k == 0), stop=(k == KO - 1),
                    )
                lgT_sb = rpool.tile([E, TCH], F32, tag="lgT_sb")
                nc.vector.tensor_copy(lgT_sb[:], lg_ps[:])
                s_ps = pps.tile([128, JO + 1, TCH], F32, tag="s_ps")
                nc.tensor.transpose(s_ps[:, JO, :E], lgT_sb[:],
                                    identity[:E, :E])
                logits = rpool.tile([128, E], F32, tag="logits")
                nc.vector.tensor_copy(logits[:], s_ps[:, JO, :E])

                if tci == 0:
                    nc.sync.dma_start(gu_sb[:, :, HB:2 * HB],
                                      gu_ap[:, :, HB:2 * HB])

                # ---- M1, b-major: one PSUM bank at a time ----
                act_sb = apool.tile([128, I], F32, tag="act")
                for b in range(3):
                    h_ps = pph.tile([128, HB], F32, tag="h_ps",
                                    name=f"h{tci}_{b}")
                    for k in range(KO):
                        nc.tensor.matmul(
                            h_ps[:], xT_c[:, k],
                            gu_sb[:, k, b * HB:(b + 1) * HB],
                            start=(k == 0), stop=(k == KO - 1),
                        )
                    if tci == 0 and b == 0:
                        nc.sync.dma_start(gu_sb[:, :, 2 * HB:I2],
                                          gu_ap[:, :, 2 * HB:I2])
                    if tci == 0 and b == 1:
                        for j in range(JO):
                            nc.sync.dma_start(dn_sb[:, j], dnT_ap[:, j])
                    # host interleave: h_b = [256 gate | 256 up]
                    silu_sb = apool.tile([128, 256], F32, tag="silu")
                    nc.scalar.activation(silu_sb[:], h_ps[:, :256],
                                         ACTF.Silu)
                    nc.vector.tensor_tensor(
                        act_sb[:, 256 * b:256 * (b + 1)], silu_sb[:],
                        h_ps[:, 256:], OP.mult,
                    )

                # ---- transpose act -> actT ----
                actT_sb = apool.tile([128, JO, TCH], F32R, tag="actT")
                for j in range(JO):
                    nc.tensor.transpose(
                        s_ps[:, j], act_sb[:, j * 128:(j + 1) * 128],
                        identity,
                    )
                    nc.vector.tensor_copy(actT_sb[:, j], s_ps[:, j])

                # ---- M2 ----
                y_pss = []
                for hb in range(H // HB):
                    y_ps = ppy.tile([128, HB], F32, tag="y_ps",
                                    name=f"y_ps{tci}_{hb}")
                    for j in range(JO):
                        nc.tensor.matmul(
                            y_ps[:], actT_sb[:, j],
                            dn_sb[:, j, hb * HB:(hb + 1) * HB],
                            start=(j == 0), stop=(j == JO - 1),
                        )
                    y_pss.append(y_ps)

                # ---- top-8 router chain (DVE) ----
                # Pin the chain after the casts so it can't hog DVE while the
                # next chunk's SwiGLU needs the h banks released. The first
                # chunk has an idle DVE (DMA-bound head) and the last has no
                # successor to protect, so let those chains run early.
                cur = rpool.tile([128, E], F32, tag="cur")
                if 0 < tci < NCHUNK - 1:
                    dep = rpool.tile([128, E], F32, tag="dep")
                    nc.vector.tensor_scalar(
                        dep[:], actT_sb[:, JO - 1, :E].bitcast(F32), 0.0,
                        None, OP.mult)
                    nc.vector.tensor_tensor(cur[:], logits[:], dep[:], OP.add)
                else:
                    nc.vector.tensor_copy(cur[:], logits[:])
                msk = rpool.tile([128, E], U8, tag="msk")
                m1 = rpool.tile([128, 1], F32, tag="m1")
                mk = rpool.tile([128, 1], F32, tag="mk")
                for it in range(TOP_K - 1):
                    tgt = m1 if it == 0 else mk
                    nc.vector.reduce_max(tgt[:], cur[:], axis=AX)
                    nc.vector.tensor_scalar(msk[:], cur[:], tgt[:],
                                            None, OP.is_ge)
                    nc.vector.copy_predicated(cur[:], msk[:], negbig[:])
                m8 = rpool.tile([128, 1], F32, tag="m8")
                nc.vector.reduce_max(m8[:], cur[:], axis=AX)

                nm1 = rpool.tile([128, 1], F32, tag="nm1")
                nc.vector.tensor_scalar(nm1[:], m1[:], -1.0, None, OP.mult)
                mask8 = rpool.tile([128, E], F32, tag="mask8")
                nc.vector.tensor_scalar(mask8[:], logits[:], m8[:],
                                        None, OP.is_ge)
                ew = rpool.tile([128, E], F32, tag="ew")
                nc.scalar.activation(ew[:], logits[:], ACTF.Exp, bias=nm1[:])
                nc.vector.tensor_tensor(ew[:], ew[:], mask8[:], OP.mult)
                s8 = rpool.tile([128, 1], F32, tag="s8")
                nc.vector.reduce_sum(s8[:], ew[:], axis=AX)
                nc.vector.tensor_tensor(ew[:], ew[:], mgb_sb[:], OP.mult)
                num = rpool.tile([128, 1], F32, tag="num")
                nc.vector.reduce_sum(num[:], ew[:], axis=AX)
                rs = rpool.tile([128, 1], F32, tag="rs")
                nc.vector.reciprocal(rs[:], s8[:])
                w_t = rpool.tile([128, 1], F32, tag="w_t")
                nc.vector.tensor_tensor(w_t[:], num[:], rs[:], OP.mult)

                # ---- scale + store ----
                for hb in range(H // HB):
                    y_sb = ypool.tile([128, HB], F32, tag="y_sb")
                    nc.vector.tensor_scalar(
                        y_sb[:], y_pss[hb][:], w_t[:], None, OP.mult,
                    )
                    nc.sync.dma_start(
                        y_d.ap()[tsl, hb * HB:(hb + 1) * HB], y_sb[:],
                    )
    nc.compile()
    _CACHED_NC = nc
    return nc


_GATEUP_PERM = np.concatenate(
    [np.r_[256 * b:256 * b + 256, 768 + 256 * b:768 + 256 * b + 256]
     for b in range(3)]
)


def prepare_in_maps(hidden_states, gate_weight, gate_up_proj, down_proj,
                    merge_groups, dominant_experts):
    x = np.asarray(hidden_states, dtype=np.float32).reshape(T, H)
    xT = np.ascontiguousarray(x.T)
    gw = np.asarray(gate_weight, dtype=np.float32)
    gwT = np.ascontiguousarray(gw.T)
    mg = np.asarray(merge_groups).astype(np.int64)
    de = np.asarray(dominant_experts).astype(np.int64)
    gup = np.asarray(gate_up_proj, dtype=np.float32)
    dnp_ = np.asarray(down_proj, dtype=np.float32)

    in_maps = []
    for g in range(G):
        e = int(de[g])
        guT = np.ascontiguousarray(gup[e].T[:, _GATEUP_PERM])
        dnT = np.ascontiguousarray(dnp_[e].T)
        mgb = np.ascontiguousarray(
            np.broadcast_to((mg == g).astype(np.float32)[None, :], (128, E))
        )
        in_maps.append({"xT": xT, "gu": guT, "gw": gwT, "dnT": dnT,
                        "mgb": mgb})
    return in_maps


def kernel(hidden_states, gate_weight, gate_up_proj, down_proj,
           merge_groups, dominant_experts):
    in_maps = prepare_in_maps(hidden_states, gate_weight, gate_up_proj,
                              down_proj, merge_groups, dominant_experts)
    nc = _build()
    res = run_bass_kernel_spmd(nc, in_maps, core_ids=list(range(G)),
                               trace=False)
    out = np.zeros((T, H), dtype=np.float64)
    for r in res.results:
        out += r["y"].astype(np.float64)
    return out.astype(np.float32).reshape(1, T, H)
